# revision 10
# baseline (speedup 1.0000x reference)
"""Trainium2 Bass kernel for nn_Block_15066745274698 (GQA attention block).

Computation (B=1, T=4096, C=2048, 16 heads x 128, 4 KV groups):
  qkv = x @ W_attn.T ; split q/k/v ; RoPE(q, k) ; causal GQA attention ;
  out = y @ W_proj.T

Sharding: head-parallel over 8 cores, 2 query heads + their KV group per
core. No collectives: each core computes a partial out^T (its 2 heads
pushed through the matching W_proj columns); the host sums the 8 partials.

Device layout (per core) is transpose-oriented so every matmul contracts
over the partition dim with zero on-device transposes of activations:
  qkv^T (f x t) = W_attn_slice^T.T @ x^T      [via lhsT = W_attn^T tiles]
  S^T   (s x t) = K^T.T @ Q^T                 [scores transposed]
  y^T   (d x t) = V.T @ exp(S^T)              [V transposed once on PE]
  out^T (o x t) = W_proj_slice^T.T @ y^T
Softmax: no max-subtraction (scores bounded ~ +-5), exp on ACT with fused
1/sqrt(128) scale, causal handled by block skipping + gpsimd affine_select
on diagonal blocks, denominators via ones-vector matmul + DMA broadcast +
fast reciprocal.
"""
import sys

sys.path.insert(0, "/opt/trn_rl_repo")
import types

import numpy as np
import ml_dtypes

import concourse.bass as bass
import concourse.mybir as mybir
import concourse.tile as tile
from concourse import bacc
from concourse.bass import ts
from concourse.bass_utils import run_bass_kernel_spmd
from concourse.masks import make_identity

T, C = 4096, 2048
HS = 128
TT = 512                 # t-tile (matmul moving free dim)
NT = T // TT             # 8
NCT = C // 128           # 16 c-tiles
F = 512                  # per-core W_attn rows: 2 q heads + k + v
SCALE = 1.0 / float(np.sqrt(np.float32(HS)))

dt = mybir.dt
FP32 = dt.float32
BF16 = dt.bfloat16
AF = mybir.ActivationFunctionType
ALU = mybir.AluOpType

_cache = {}


def install_ntff_hook_shim():
    """antenv.axon_hooks is missing from this image; register the
    ctypes-based NTFF hook ourselves so trace=True works under axon."""
    if "antenv.axon_hooks" in sys.modules:
        return
    import antenv

    mod = types.ModuleType("antenv.axon_hooks")
    mod._hook = None
    mod.set_axon_ntff_profile_hook = lambda h: setattr(mod, "_hook", h)
    mod.get_axon_ntff_profile_hook = lambda: mod._hook
    sys.modules["antenv.axon_hooks"] = mod
    antenv.axon_hooks = mod
    try:
        from trn_agent_boot.trn_boot import _ntff_profile_via_ctypes

        mod.set_axon_ntff_profile_hook(
            _ntff_profile_via_ctypes("/opt/axon/libaxon_pjrt.so")
        )
    except Exception:
        pass


def _rope(nc, rtmp, cos_sl, sin_sl, src_ps, dst):
    """Rotate-half RoPE: src_ps (128d x TT) psum fp32 -> dst (128 x TT) bf16.
    cos_sl/sin_sl are (128 x TT) fp32 with the 64 rotary rows duplicated.
    Two-input DVE ops need equal base partitions only when BOTH inputs are
    SBUF; src_ps is PSUM, so the rotate-half partition shift is applied on
    the PSUM operand and all SBUF+SBUF pairs stay base-aligned."""
    tcos = rtmp.tile([128, TT], FP32, tag="tcos")
    tsin = rtmp.tile([128, TT], FP32, tag="tsin")
    nc.vector.tensor_mul(tcos, src_ps, cos_sl)
    nc.vector.tensor_mul(tsin[0:64, :], src_ps[64:128, :], sin_sl[0:64, :])
    nc.vector.tensor_mul(tsin[64:128, :], src_ps[0:64, :], sin_sl[64:128, :])
    nc.vector.tensor_sub(dst[0:64, :], tcos[0:64, :], tsin[0:64, :])
    nc.vector.tensor_add(dst[64:128, :], tcos[64:128, :], tsin[64:128, :])


def build(taps=False):
    nc = bacc.Bacc(
        "TRN2", target_bir_lowering=False, debug=False, enable_asserts=False
    )
    xT = nc.dram_tensor("xT", [C, T], BF16, kind="ExternalInput").ap()
    waT = nc.dram_tensor("waT", [C, F], BF16, kind="ExternalInput").ap()
    wpT = nc.dram_tensor("wpT", [2 * HS, C], BF16, kind="ExternalInput").ap()
    cos2 = nc.dram_tensor("cos2", [128, T], FP32, kind="ExternalInput").ap()
    sin2 = nc.dram_tensor("sin2", [128, T], FP32, kind="ExternalInput").ap()
    outT = nc.dram_tensor("outT", [C, T], FP32, kind="ExternalOutput").ap()
    if taps:
        d_qkvT = nc.dram_tensor("d_qkvT", [F, T], FP32, kind="ExternalOutput").ap()
        d_QT = nc.dram_tensor("d_QT", [256, T], BF16, kind="ExternalOutput").ap()
        d_KT = nc.dram_tensor("d_KT", [128, T], BF16, kind="ExternalOutput").ap()
        d_V = nc.dram_tensor("d_V", [T, 128], BF16, kind="ExternalOutput").ap()
        d_den = nc.dram_tensor("d_den", [2, T], FP32, kind="ExternalOutput").ap()
        d_y = nc.dram_tensor("d_y", [256, T], BF16, kind="ExternalOutput").ap()
        d_P = nc.dram_tensor("d_P", [T, TT], BF16, kind="ExternalOutput").ap()

    xT_r = xT.rearrange("(a p) t -> p a t", p=128)     # [128, 16, 4096]
    waT_r = waT.rearrange("(a p) f -> p a f", p=128)   # [128, 16, 512]
    wpT_r = wpT.rearrange("(a p) o -> p a o", p=128)   # [128, 2, 2048]

    with tile.TileContext(nc) as tc:
        with (
            tc.tile_pool(name="singles", bufs=1) as singles,
            tc.tile_pool(name="xp", bufs=2) as xp,
            tc.tile_pool(name="qp", bufs=2 * NT) as qp,
            tc.tile_pool(name="kp", bufs=NT) as kp,
            tc.tile_pool(name="vp", bufs=4 * NT) as vp,
            tc.tile_pool(name="vstage", bufs=2) as vstage,
            tc.tile_pool(name="pp", bufs=4) as pp,
            tc.tile_pool(name="rtmp", bufs=2) as rtmp,
            tc.tile_pool(name="ysb", bufs=3) as ysb,
            tc.tile_pool(name="rbp", bufs=2) as rbp,
            tc.tile_pool(name="osb", bufs=3) as osb,
            tc.tile_pool(name="mm_ps", bufs=2, space="PSUM") as mm_ps,
            tc.tile_pool(name="s_ps", bufs=2, space="PSUM") as s_ps,
            tc.tile_pool(name="y_ps", bufs=2, space="PSUM") as y_ps,
            tc.tile_pool(name="aux_ps", bufs=1, space="PSUM") as aux_ps,
        ):
            # ---- persistent tiles ----
            wa_sb = singles.tile([128, NCT, F], BF16)
            nc.sync.dma_start(wa_sb, waT_r)
            wp_sb = singles.tile([128, 2, C], BF16)
            nc.sync.dma_start(wp_sb, wpT_r)
            cos_sb = singles.tile([128, T], FP32)
            nc.sync.dma_start(cos_sb, cos2)
            sin_sb = singles.tile([128, T], FP32)
            nc.sync.dma_start(sin_sb, sin2)
            ident = singles.tile([128, 128], BF16)
            make_identity(nc, ident)
            ones_sb = singles.tile([128, 1], BF16)
            nc.vector.memset(ones_sb, 1.0)

            q_tiles = [[None] * NT for _ in range(2)]
            k_tiles = [None] * NT
            v_tiles = [None] * (4 * NT)

            for i in range(NT):
                # ---- QKV projection for t-chunk i ----
                xt = xp.tile([128, NCT, TT], BF16, tag="xt")
                nc.sync.dma_start(xt, xT_r[:, :, ts(i, TT)])
                for f in range(4):
                    ps = mm_ps.tile([128, TT], FP32, tag="mm")
                    for ci in range(NCT):
                        nc.tensor.matmul(
                            ps,
                            wa_sb[:, ci, f * 128:(f + 1) * 128],
                            xt[:, ci, :],
                            start=(ci == 0),
                            stop=(ci == NCT - 1),
                        )
                    if taps:
                        dbg = osb.tile([128, TT], FP32, tag="dbg")
                        nc.vector.tensor_copy(dbg, ps)
                        nc.sync.dma_start(
                            d_qkvT[f * 128:(f + 1) * 128, ts(i, TT)], dbg
                        )
                    if f < 2:
                        dst = qp.tile([128, TT], BF16, tag="qt")
                        q_tiles[f][i] = dst
                        _rope(nc, rtmp, cos_sb[:, ts(i, TT)],
                              sin_sb[:, ts(i, TT)], ps, dst)
                        if taps:
                            nc.sync.dma_start(
                                d_QT[f * 128:(f + 1) * 128, ts(i, TT)], dst
                            )
                    elif f == 2:
                        dst = kp.tile([128, TT], BF16, tag="kt")
                        k_tiles[i] = dst
                        _rope(nc, rtmp, cos_sb[:, ts(i, TT)],
                              sin_sb[:, ts(i, TT)], ps, dst)
                        if taps:
                            nc.sync.dma_start(d_KT[:, ts(i, TT)], dst)
                    else:
                        vst = vstage.tile([128, TT], BF16, tag="vst")
                        nc.vector.tensor_copy(vst, ps)
                        for j4 in range(4):
                            tp = aux_ps.tile([128, 128], BF16, tag="tp")
                            nc.tensor.transpose(
                                tp, vst[:, j4 * 128:(j4 + 1) * 128], ident
                            )
                            vt = vp.tile([128, 128], BF16, tag="vt")
                            v_tiles[i * 4 + j4] = vt
                            nc.vector.tensor_copy(vt, tp)
                            if taps:
                                nc.sync.dma_start(
                                    d_V[(i * 4 + j4) * 128:(i * 4 + j4 + 1) * 128, :],
                                    vt,
                                )

                # ---- attention for t-chunk i, both heads ----
                yts = []
                ns = 4 * (i + 1)
                for h in range(2):
                    yp = y_ps.tile([128, TT], FP32, tag="y")
                    dp = aux_ps.tile([1, TT], FP32, tag="den")
                    for j in range(ns):
                        sp = s_ps.tile([128, TT], FP32, tag="s")
                        nc.tensor.matmul(
                            sp,
                            k_tiles[j // 4][:, (j % 4) * 128:(j % 4 + 1) * 128],
                            q_tiles[h][i],
                            start=True,
                            stop=True,
                        )
                        p_sb = pp.tile([128, TT], BF16, tag="p")
                        nc.scalar.activation(p_sb, sp, AF.Exp, scale=SCALE)
                        if j >= 4 * i:
                            # diagonal block: zero entries with s > t, i.e.
                            # keep iff (t0 + y) - (s0 + p) >= 0; the iota is
                            # base + channel_multiplier*p + step*y, compared
                            # against 0 by compare_op.
                            nc.gpsimd.affine_select(
                                out=p_sb,
                                in_=p_sb,
                                compare_op=ALU.is_ge,
                                fill=0.0,
                                base=i * TT - j * 128,
                                pattern=[[1, TT]],
                                channel_multiplier=-1,
                            )
                        nc.tensor.matmul(
                            yp, v_tiles[j], p_sb,
                            start=(j == 0), stop=(j == ns - 1),
                        )
                        nc.tensor.matmul(
                            dp, ones_sb, p_sb,
                            start=(j == 0), stop=(j == ns - 1),
                        )
                        if taps and h == 0 and i == NT - 1:
                            nc.sync.dma_start(
                                d_P[j * 128:(j + 1) * 128, :], p_sb
                            )
                    # softmax denominator -> reciprocal, broadcast to 128 rows
                    drow = rbp.tile([1, TT], FP32, tag="drow")
                    nc.vector.tensor_copy(drow, dp)
                    db = rbp.tile([128, TT], FP32, tag="db")
                    nc.gpsimd.partition_broadcast(db, drow)
                    rb = rbp.tile([128, TT], FP32, tag="rb")
                    nc.vector.reciprocal_approx_fast(out=rb, in_=db)
                    yt = ysb.tile([128, TT], BF16, tag="yt")
                    nc.vector.tensor_mul(yt, yp, rb)
                    yts.append(yt)
                    if taps:
                        nc.sync.dma_start(d_den[h:h + 1, ts(i, TT)], drow)
                        nc.sync.dma_start(
                            d_y[h * 128:(h + 1) * 128, ts(i, TT)], yt
                        )

                # ---- out projection for t-chunk i ----
                for oi in range(NCT):
                    op = mm_ps.tile([128, TT], FP32, tag="mm")
                    for cj in range(2):
                        nc.tensor.matmul(
                            op,
                            wp_sb[:, cj, oi * 128:(oi + 1) * 128],
                            yts[cj],
                            start=(cj == 0),
                            stop=(cj == 1),
                        )
                    ot = osb.tile([128, TT], FP32, tag="ot")
                    nc.vector.tensor_copy(ot, op)
                    nc.sync.dma_start(
                        outT[oi * 128:(oi + 1) * 128, ts(i, TT)], ot
                    )

    nc.compile()
    return nc


def _prep_inputs(x, cos, sin, W_attn, W_proj):
    bf = ml_dtypes.bfloat16
    x = np.asarray(x, dtype=np.float32)
    cos = np.asarray(cos, dtype=np.float32)
    sin = np.asarray(sin, dtype=np.float32)
    W_attn = np.asarray(W_attn, dtype=np.float32)
    W_proj = np.asarray(W_proj, dtype=np.float32)

    xT = np.ascontiguousarray(x.reshape(T, C).T).astype(bf)
    cos2 = np.ascontiguousarray(np.concatenate([cos.T, cos.T], axis=0))
    sin2 = np.ascontiguousarray(np.concatenate([sin.T, sin.T], axis=0))

    in_maps = []
    for core in range(8):
        g = core // 2
        qoff = g * 768 + (core % 2) * 256
        rows = np.concatenate(
            [
                W_attn[qoff:qoff + 256],
                W_attn[g * 768 + 512:g * 768 + 640],
                W_attn[g * 768 + 640:g * 768 + 768],
            ],
            axis=0,
        )
        waT = np.ascontiguousarray(rows.T).astype(bf)
        h0 = g * 4 + (core % 2) * 2
        wpT = np.ascontiguousarray(W_proj[:, h0 * 128:h0 * 128 + 256].T).astype(bf)
        in_maps.append(
            {"xT": xT, "waT": waT, "wpT": wpT, "cos2": cos2, "sin2": sin2}
        )
    return in_maps


def kernel(x, cos, sin, W_attn, W_proj, _trace=False, _trace_cores=None):
    if "nc" not in _cache:
        _cache["nc"] = build()
    nc = _cache["nc"]
    in_maps = _prep_inputs(x, cos, sin, W_attn, W_proj)
    kwargs = {}
    if _trace:
        install_ntff_hook_shim()
        kwargs = dict(trace=True, trace_cores=_trace_cores or [0])
    res = run_bass_kernel_spmd(nc, in_maps, core_ids=list(range(8)), **kwargs)
    acc = np.zeros((C, T), dtype=np.float32)
    for r in res.results:
        acc += r["outT"]
    out = np.ascontiguousarray(acc.T).reshape(1, T, C)
    _cache["last_results"] = res
    return out


# revision 52
# speedup vs baseline: 1.1627x; 1.1627x over previous
"""Trainium2 Bass kernel for nn_Block_15066745274698 (GQA attention block).

Computation (B=1, T=4096, C=2048, 16 heads x 128, 4 KV groups):
  qkv = x @ W_attn.T ; split q/k/v ; RoPE(q, k) ; causal GQA attention ;
  out = y @ W_proj.T

Sharding: head-parallel over 8 cores, 2 query heads + their KV group per
core. No collectives: each core computes a partial out^T (its 2 heads
pushed through the matching W_proj columns); the host sums the 8 partials.

Device layout (per core) is transpose-oriented so every matmul contracts
over the partition dim with zero on-device transposes of activations:
  qkv^T (f x t) = W_attn_slice^T.T @ x^T      [via lhsT = W_attn^T tiles]
  S^T   (s x t) = K^T.T @ Q^T                 [scores transposed]
  y^T   (d x t) = V.T @ exp(S^T)              [V transposed once on PE]
  out^T (o x t) = W_proj_slice^T.T @ y^T
Softmax: no max-subtraction (scores bounded ~ +-5), exp on ACT with fused
1/sqrt(128) scale, causal handled by block skipping + gpsimd affine_select
on diagonal blocks, denominators via ones-vector matmul + DMA broadcast +
fast reciprocal.
"""
import sys

sys.path.insert(0, "/opt/trn_rl_repo")
import types

import numpy as np
import ml_dtypes

import os

import concourse.bass as bass
import concourse.mybir as mybir
import concourse.tile as tile
from concourse import bacc
from concourse.bass import ts
from concourse.bass_utils import run_bass_kernel_spmd
from concourse.masks import make_identity

if os.environ.get("K_LDWOPT", "0") == "1":
    import concourse.bass_utils as _bu

    _orig_run_command = _bu.run_command

    def _patched_run_command(argv, **kwargs):
        argv = [
            a.replace("--enable-ldw-opt=false", "--enable-ldw-opt=true")
            if isinstance(a, str)
            else a
            for a in argv
        ]
        return _orig_run_command(argv, **kwargs)

    _bu.run_command = _patched_run_command

T, C = 4096, 2048
HS = 128
TT = 512                 # t-tile (matmul moving free dim)
NT = T // TT             # 8
NCT = C // 128           # 16 c-tiles
F = 512                  # per-core W_attn rows: 2 q heads + k + v
SCALE = 1.0 / float(np.sqrt(np.float32(HS)))

dt = mybir.dt
FP32 = dt.float32
BF16 = dt.bfloat16
AF = mybir.ActivationFunctionType
ALU = mybir.AluOpType

_cache = {}


def install_ntff_hook_shim():
    """antenv.axon_hooks is missing from this image; register the
    ctypes-based NTFF hook ourselves so trace=True works under axon."""
    if "antenv.axon_hooks" in sys.modules:
        return
    import antenv

    mod = types.ModuleType("antenv.axon_hooks")
    mod._hook = None
    mod.set_axon_ntff_profile_hook = lambda h: setattr(mod, "_hook", h)
    mod.get_axon_ntff_profile_hook = lambda: mod._hook
    sys.modules["antenv.axon_hooks"] = mod
    antenv.axon_hooks = mod
    try:
        from trn_agent_boot.trn_boot import _ntff_profile_via_ctypes

        mod.set_axon_ntff_profile_hook(
            _ntff_profile_via_ctypes("/opt/axon/libaxon_pjrt.so")
        )
    except Exception:
        pass


def _rope(nc, rtmp, cos_sl, sin_sl, src_ps, dst):
    """Rotate-half RoPE: src_ps (128d x TT) psum fp32 -> dst (128 x TT) bf16.
    cos_sl/sin_sl are (128 x TT) fp32 with the 64 rotary rows duplicated.
    Two-input DVE ops need equal base partitions only when BOTH inputs are
    SBUF; src_ps is PSUM, so the rotate-half partition shift is applied on
    the PSUM operand and all SBUF+SBUF pairs stay base-aligned."""
    tcos = rtmp.tile([128, TT], FP32, tag="tcos")
    tsin = rtmp.tile([128, TT], FP32, tag="tsin")
    nc.vector.tensor_mul(tcos, src_ps, cos_sl)
    nc.vector.tensor_mul(tsin[0:64, :], src_ps[64:128, :], sin_sl[0:64, :])
    nc.vector.tensor_mul(tsin[64:128, :], src_ps[0:64, :], sin_sl[64:128, :])
    nc.vector.tensor_sub(dst[0:64, :], tcos[0:64, :], tsin[0:64, :])
    nc.vector.tensor_add(dst[64:128, :], tcos[64:128, :], tsin[64:128, :])


def build(taps=False):
    nc = bacc.Bacc(
        "TRN2", target_bir_lowering=False, debug=False, enable_asserts=False
    )
    xT = nc.dram_tensor("xT", [C, T], BF16, kind="ExternalInput").ap()
    waT = nc.dram_tensor("waT", [C, F], BF16, kind="ExternalInput").ap()
    wpT = nc.dram_tensor("wpT", [2 * HS, C], BF16, kind="ExternalInput").ap()
    cos2 = nc.dram_tensor("cos2", [128, T], FP32, kind="ExternalInput").ap()
    sin2 = nc.dram_tensor("sin2", [128, T], FP32, kind="ExternalInput").ap()
    outT = nc.dram_tensor("outT", [C, T], FP32, kind="ExternalOutput").ap()
    if taps:
        d_qkvT = nc.dram_tensor("d_qkvT", [F, T], FP32, kind="ExternalOutput").ap()
        d_QT = nc.dram_tensor("d_QT", [256, T], BF16, kind="ExternalOutput").ap()
        d_KT = nc.dram_tensor("d_KT", [128, T], BF16, kind="ExternalOutput").ap()
        d_V = nc.dram_tensor("d_V", [T, 128], BF16, kind="ExternalOutput").ap()
        d_den = nc.dram_tensor("d_den", [2, T], FP32, kind="ExternalOutput").ap()
        d_y = nc.dram_tensor("d_y", [256, T], BF16, kind="ExternalOutput").ap()
        d_P = nc.dram_tensor("d_P", [T, TT], BF16, kind="ExternalOutput").ap()

    xT_r = xT.rearrange("(a p) t -> p a t", p=128)     # [128, 16, 4096]
    waT_r = waT.rearrange("(a p) f -> p a f", p=128)   # [128, 16, 512]
    wpT_r = wpT.rearrange("(a p) o -> p a o", p=128)   # [128, 2, 2048]

    with tile.TileContext(nc) as tc:
        with (
            tc.tile_pool(name="singles", bufs=1) as singles,
            tc.tile_pool(name="xp", bufs=3) as xp,
            tc.tile_pool(name="qp", bufs=2 * NT) as qp,
            tc.tile_pool(name="kp", bufs=NT) as kp,
            tc.tile_pool(name="vp", bufs=4 * NT) as vp,
            tc.tile_pool(name="vstage", bufs=3) as vstage,
            tc.tile_pool(name="pp", bufs=10) as pp,
            tc.tile_pool(name="rtmp", bufs=6) as rtmp,
            tc.tile_pool(name="ysb", bufs=6) as ysb,
            tc.tile_pool(name="rbp", bufs=2) as rbp,
            tc.tile_pool(name="osb", bufs=6) as osb,
            tc.tile_pool(name="mm_ps", bufs=3, space="PSUM") as mm_ps,
            tc.tile_pool(name="s_ps", bufs=2, space="PSUM") as s_ps,
            tc.tile_pool(name="y_ps", bufs=2, space="PSUM") as y_ps,
            tc.tile_pool(name="aux_ps", bufs=1, space="PSUM") as aux_ps,
        ):
            # ---- persistent tiles (DMA order matters: the very first qkv
            # matmuls need wa chunk 0 + x chunk 0; cos/sin follow for RoPE;
            # wp is not needed until the first out-projection) ----
            wa_sb = singles.tile([128, NCT, F], BF16)
            xt0 = xp.tile([128, NCT, TT], BF16, tag="xt")
            # first c-chunks as small separate transfers on two queues so the
            # first qkv matmul can start within a couple of microseconds
            nc.sync.dma_start(wa_sb[:, 0:2, :], waT_r[:, 0:2, :])
            nc.scalar.dma_start(xt0[:, 0:2, :], xT_r[:, 0:2, 0:TT])
            nc.sync.dma_start(wa_sb[:, 2:4, :], waT_r[:, 2:4, :])
            nc.scalar.dma_start(xt0[:, 2:4, :], xT_r[:, 2:4, 0:TT])
            for q in range(1, 4):
                nc.sync.dma_start(
                    wa_sb[:, 4 * q:4 * (q + 1), :], waT_r[:, 4 * q:4 * (q + 1), :]
                )
                nc.scalar.dma_start(
                    xt0[:, 4 * q:4 * (q + 1), :],
                    xT_r[:, 4 * q:4 * (q + 1), 0:TT],
                )
            cos_sb = singles.tile([128, T], FP32)
            nc.scalar.dma_start(cos_sb, cos2)
            sin_sb = singles.tile([128, T], FP32)
            nc.scalar.dma_start(sin_sb, sin2)
            wp_sb = singles.tile([128, 2, C], BF16)
            nc.gpsimd.dma_start(wp_sb, wpT_r)
            ident = singles.tile([128, 128], BF16)
            make_identity(nc, ident)
            ones_sb = singles.tile([128, 1], BF16)
            nc.vector.memset(ones_sb, 1.0)
            F32R = dt.float32r
            ones_colf = singles.tile([1, 128], FP32)
            nc.vector.memset(ones_colf, 1.0)
            ones_col = singles.tile([1, 128], F32R)
            nc.vector.tensor_copy(ones_col, ones_colf)

            q_tiles = [[None] * NT for _ in range(2)]
            k_tiles = [None] * NT
            v_tiles = [None] * (4 * NT)
            y_chunks = [[] for _ in range(NT)]

            def emit_proj(i):
                # out projection for t-chunk i; emitted one t-tile late so
                # attention matmuls are available to fill PE stalls while the
                # DVE/ACT drain copies pace the psum slot rotation
                for oi in range(NCT):
                    op = mm_ps.tile([128, TT], FP32, tag="mm")
                    for cj in range(2):
                        nc.tensor.matmul(
                            op,
                            wp_sb[:, cj, oi * 128:(oi + 1) * 128],
                            y_chunks[i][cj],
                            start=(cj == 0),
                            stop=(cj == 1),
                        )
                    ot = osb.tile([128, TT], FP32, tag="ot")
                    if oi % 2 == 0:
                        nc.vector.tensor_copy(ot, op)
                    else:
                        nc.scalar.copy(ot, op)
                    nc.sync.dma_start(
                        outT[oi * 128:(oi + 1) * 128, ts(i, TT)], ot
                    )

            for i in range(NT):
                # ---- QKV projection for t-chunk i ----
                if i == 0:
                    xt = xt0
                else:
                    xt = xp.tile([128, NCT, TT], BF16, tag="xt")
                    for q in range(4):
                        nc.sync.dma_start(
                            xt[:, 4 * q:4 * (q + 1), :],
                            xT_r[:, 4 * q:4 * (q + 1), ts(i, TT)],
                        )
                for f in range(4):
                    ps = mm_ps.tile([128, TT], FP32, tag="mm")
                    for ci in range(NCT):
                        nc.tensor.matmul(
                            ps,
                            wa_sb[:, ci, f * 128:(f + 1) * 128],
                            xt[:, ci, :],
                            start=(ci == 0),
                            stop=(ci == NCT - 1),
                        )
                    if taps:
                        dbg = osb.tile([128, TT], FP32, tag="dbg")
                        nc.vector.tensor_copy(dbg, ps)
                        nc.sync.dma_start(
                            d_qkvT[f * 128:(f + 1) * 128, ts(i, TT)], dbg
                        )
                    if f < 2:
                        dst = qp.tile([128, TT], BF16, tag="qt")
                        q_tiles[f][i] = dst
                        _rope(nc, rtmp, cos_sb[:, ts(i, TT)],
                              sin_sb[:, ts(i, TT)], ps, dst)
                        if taps:
                            nc.sync.dma_start(
                                d_QT[f * 128:(f + 1) * 128, ts(i, TT)], dst
                            )
                    elif f == 2:
                        dst = kp.tile([128, TT], BF16, tag="kt")
                        k_tiles[i] = dst
                        _rope(nc, rtmp, cos_sb[:, ts(i, TT)],
                              sin_sb[:, ts(i, TT)], ps, dst)
                        if taps:
                            nc.sync.dma_start(d_KT[:, ts(i, TT)], dst)
                    else:
                        vst = vstage.tile([128, TT], BF16, tag="vst")
                        nc.vector.tensor_copy(vst, ps)
                        for j4 in range(4):
                            tp = mm_ps.tile([128, 128], BF16, tag="mm")
                            nc.tensor.transpose(
                                tp, vst[:, j4 * 128:(j4 + 1) * 128], ident
                            )
                            vt = vp.tile([128, 128], BF16, tag="vt")
                            v_tiles[i * 4 + j4] = vt
                            nc.vector.tensor_copy(vt, tp)
                            if taps:
                                nc.sync.dma_start(
                                    d_V[(i * 4 + j4) * 128:(i * 4 + j4 + 1) * 128, :],
                                    vt,
                                )

                # ---- attention for t-chunk i, both heads ----
                yts = y_chunks[i]
                ns = 4 * (i + 1)
                for h in range(2):
                    yp = y_ps.tile([128, TT], FP32, tag="y")
                    dp = aux_ps.tile([1, TT], FP32, tag="den")
                    for j in range(ns):
                        sp = s_ps.tile([128, TT], FP32, tag="s")
                        nc.tensor.matmul(
                            sp,
                            k_tiles[j // 4][:, (j % 4) * 128:(j % 4 + 1) * 128],
                            q_tiles[h][i],
                            start=True,
                            stop=True,
                        )
                        p_sb = pp.tile([128, TT], BF16, tag="p")
                        nc.scalar.activation(p_sb, sp, AF.Exp, scale=SCALE)
                        if j >= 4 * i:
                            # diagonal block: zero entries with s > t, i.e.
                            # keep iff (t0 + y) - (s0 + p) >= 0; the iota is
                            # base + channel_multiplier*p + step*y, compared
                            # against 0 by compare_op.
                            nc.gpsimd.affine_select(
                                out=p_sb,
                                in_=p_sb,
                                compare_op=ALU.is_ge,
                                fill=0.0,
                                base=i * TT - j * 128,
                                pattern=[[1, TT]],
                                channel_multiplier=-1,
                            )
                        nc.tensor.matmul(
                            yp, v_tiles[j], p_sb,
                            start=(j == 0), stop=(j == ns - 1),
                        )
                        nc.tensor.matmul(
                            dp, ones_sb, p_sb,
                            start=(j == 0), stop=(j == ns - 1),
                        )
                        if taps and h == 0 and i == NT - 1:
                            nc.sync.dma_start(
                                d_P[j * 128:(j + 1) * 128, :], p_sb
                            )
                    # softmax denominator -> broadcast to 128 rows via a tiny
                    # K=1 fp32r matmul on PE (gpsimd partition_broadcast is
                    # ~1.7us + heavy sem cost on this path), then reciprocal
                    drow = rbp.tile([1, TT], F32R, tag="drow")
                    nc.vector.tensor_copy(drow, dp)
                    bc = mm_ps.tile([128, TT], FP32, tag="mm")
                    nc.tensor.matmul(bc, ones_col, drow, start=True, stop=True)
                    rb = rbp.tile([128, TT], FP32, tag="rb")
                    nc.vector.reciprocal_approx_fast(out=rb, in_=bc)
                    yt = ysb.tile([128, TT], BF16, tag="yt")
                    nc.vector.tensor_mul(yt, yp, rb)
                    yts.append(yt)
                    if taps:
                        nc.sync.dma_start(d_den[h:h + 1, ts(i, TT)], drow)
                        nc.sync.dma_start(
                            d_y[h * 128:(h + 1) * 128, ts(i, TT)], yt
                        )

                if i > 0:
                    emit_proj(i - 1)
            emit_proj(NT - 1)

    nc.compile()
    return nc


def _prep_inputs(x, cos, sin, W_attn, W_proj):
    bf = ml_dtypes.bfloat16
    x = np.asarray(x, dtype=np.float32)
    cos = np.asarray(cos, dtype=np.float32)
    sin = np.asarray(sin, dtype=np.float32)
    W_attn = np.asarray(W_attn, dtype=np.float32)
    W_proj = np.asarray(W_proj, dtype=np.float32)

    xT = np.ascontiguousarray(x.reshape(T, C).T).astype(bf)
    cos2 = np.ascontiguousarray(np.concatenate([cos.T, cos.T], axis=0))
    sin2 = np.ascontiguousarray(np.concatenate([sin.T, sin.T], axis=0))

    in_maps = []
    for core in range(8):
        g = core // 2
        qoff = g * 768 + (core % 2) * 256
        rows = np.concatenate(
            [
                W_attn[qoff:qoff + 256],
                W_attn[g * 768 + 512:g * 768 + 640],
                W_attn[g * 768 + 640:g * 768 + 768],
            ],
            axis=0,
        )
        waT = np.ascontiguousarray(rows.T).astype(bf)
        h0 = g * 4 + (core % 2) * 2
        wpT = np.ascontiguousarray(W_proj[:, h0 * 128:h0 * 128 + 256].T).astype(bf)
        in_maps.append(
            {"xT": xT, "waT": waT, "wpT": wpT, "cos2": cos2, "sin2": sin2}
        )
    return in_maps


def kernel(x, cos, sin, W_attn, W_proj, _trace=False, _trace_cores=None):
    if "nc" not in _cache:
        _cache["nc"] = build()
    nc = _cache["nc"]
    in_maps = _prep_inputs(x, cos, sin, W_attn, W_proj)
    kwargs = {}
    if _trace:
        install_ntff_hook_shim()
        kwargs = dict(trace=True, trace_cores=_trace_cores or [0])
    res = run_bass_kernel_spmd(nc, in_maps, core_ids=list(range(8)), **kwargs)
    acc = np.zeros((C, T), dtype=np.float32)
    for r in res.results:
        acc += r["outT"]
    out = np.ascontiguousarray(acc.T).reshape(1, T, C)
    _cache["last_results"] = res
    return out


# revision 57
# speedup vs baseline: 1.3101x; 1.1268x over previous
"""Trainium2 Bass kernel for nn_Block_15066745274698 (GQA attention block).

Computation (B=1, T=4096, C=2048, 16 heads x 128, 4 KV groups):
  qkv = x @ W_attn.T ; split q/k/v ; RoPE(q, k) ; causal GQA attention ;
  out = y @ W_proj.T

Sharding: head-parallel over 8 cores, 2 query heads + their KV group per
core. No collectives: each core computes a partial out^T (its 2 heads
pushed through the matching W_proj columns); the host sums the 8 partials.

Device layout (per core) is transpose-oriented so every matmul contracts
over the partition dim with zero on-device transposes of activations:
  qkv^T (f x t) = W_attn_slice^T.T @ x^T      [via lhsT = W_attn^T tiles]
  S^T   (s x t) = K^T.T @ Q^T                 [scores transposed]
  y^T   (d x t) = V.T @ exp(S^T)              [V transposed once on PE]
  out^T (o x t) = W_proj_slice^T.T @ y^T
Softmax: no max-subtraction (scores bounded ~ +-5), exp on ACT with fused
1/sqrt(128) scale, causal handled by block skipping + gpsimd affine_select
on diagonal blocks, denominators via ones-vector matmul + DMA broadcast +
fast reciprocal.
"""
import sys

sys.path.insert(0, "/opt/trn_rl_repo")
import types

import numpy as np
import ml_dtypes

import os

import concourse.bass as bass
import concourse.mybir as mybir
import concourse.tile as tile
from concourse import bacc
from concourse.bass import ts
from concourse.bass_utils import run_bass_kernel_spmd
from concourse.masks import make_identity

if os.environ.get("K_LDWOPT", "0") == "1":
    import concourse.bass_utils as _bu

    _orig_run_command = _bu.run_command

    def _patched_run_command(argv, **kwargs):
        argv = [
            a.replace("--enable-ldw-opt=false", "--enable-ldw-opt=true")
            if isinstance(a, str)
            else a
            for a in argv
        ]
        return _orig_run_command(argv, **kwargs)

    _bu.run_command = _patched_run_command

T, C = 4096, 2048
HS = 128
TT = 512                 # t-tile (matmul moving free dim)
NT = T // TT             # 8
NCT = C // 128           # 16 c-tiles
F = 512                  # per-core W_attn rows: 2 q heads + k + v
SCALE = 1.0 / float(np.sqrt(np.float32(HS)))

dt = mybir.dt
FP32 = dt.float32
BF16 = dt.bfloat16
AF = mybir.ActivationFunctionType
ALU = mybir.AluOpType

_cache = {}


def install_ntff_hook_shim():
    """antenv.axon_hooks is missing from this image; register the
    ctypes-based NTFF hook ourselves so trace=True works under axon."""
    if "antenv.axon_hooks" in sys.modules:
        return
    import antenv

    mod = types.ModuleType("antenv.axon_hooks")
    mod._hook = None
    mod.set_axon_ntff_profile_hook = lambda h: setattr(mod, "_hook", h)
    mod.get_axon_ntff_profile_hook = lambda: mod._hook
    sys.modules["antenv.axon_hooks"] = mod
    antenv.axon_hooks = mod
    try:
        from trn_agent_boot.trn_boot import _ntff_profile_via_ctypes

        mod.set_axon_ntff_profile_hook(
            _ntff_profile_via_ctypes("/opt/axon/libaxon_pjrt.so")
        )
    except Exception:
        pass


def _rope(nc, rtmp, cos_sl, sin_sl, src_ps, dst):
    """Rotate-half RoPE: src_ps (128d x TT) psum fp32 -> dst (128 x TT) bf16.
    cos_sl/sin_sl are (128 x TT) fp32 with the 64 rotary rows duplicated.
    Two-input DVE ops need equal base partitions only when BOTH inputs are
    SBUF; src_ps is PSUM, so the rotate-half partition shift is applied on
    the PSUM operand and all SBUF+SBUF pairs stay base-aligned."""
    tcos = rtmp.tile([128, TT], FP32, tag="tcos")
    tsin = rtmp.tile([128, TT], FP32, tag="tsin")
    nc.vector.tensor_mul(tcos, src_ps, cos_sl)
    nc.vector.tensor_mul(tsin[0:64, :], src_ps[64:128, :], sin_sl[0:64, :])
    nc.vector.tensor_mul(tsin[64:128, :], src_ps[0:64, :], sin_sl[64:128, :])
    nc.vector.tensor_sub(dst[0:64, :], tcos[0:64, :], tsin[0:64, :])
    nc.vector.tensor_add(dst[64:128, :], tcos[64:128, :], tsin[64:128, :])


def build(taps=False):
    nc = bacc.Bacc(
        "TRN2", target_bir_lowering=False, debug=False, enable_asserts=False
    )
    xT = nc.dram_tensor("xT", [C, T], BF16, kind="ExternalInput").ap()
    waT = nc.dram_tensor("waT", [C, F], BF16, kind="ExternalInput").ap()
    wpT = nc.dram_tensor("wpT", [2 * HS, C], BF16, kind="ExternalInput").ap()
    cos2 = nc.dram_tensor("cos2", [128, T], FP32, kind="ExternalInput").ap()
    sin2 = nc.dram_tensor("sin2", [128, T], FP32, kind="ExternalInput").ap()
    outT = nc.dram_tensor("outT", [C, T], FP32, kind="ExternalOutput").ap()
    if taps:
        d_qkvT = nc.dram_tensor("d_qkvT", [F, T], FP32, kind="ExternalOutput").ap()
        d_QT = nc.dram_tensor("d_QT", [256, T], BF16, kind="ExternalOutput").ap()
        d_KT = nc.dram_tensor("d_KT", [128, T], BF16, kind="ExternalOutput").ap()
        d_V = nc.dram_tensor("d_V", [T, 128], BF16, kind="ExternalOutput").ap()
        d_den = nc.dram_tensor("d_den", [2, T], FP32, kind="ExternalOutput").ap()
        d_y = nc.dram_tensor("d_y", [256, T], BF16, kind="ExternalOutput").ap()
        d_P = nc.dram_tensor("d_P", [T, TT], BF16, kind="ExternalOutput").ap()

    xT_r = xT.rearrange("(a p) t -> p a t", p=128)     # [128, 16, 4096]
    waT_r = waT.rearrange("(a p) f -> p a f", p=128)   # [128, 16, 512]
    wpT_r = wpT.rearrange("(a p) o -> p a o", p=128)   # [128, 2, 2048]

    with tile.TileContext(nc) as tc:
        with (
            tc.tile_pool(name="singles", bufs=1) as singles,
            tc.tile_pool(name="xp", bufs=3) as xp,
            tc.tile_pool(name="qp", bufs=2 * NT) as qp,
            tc.tile_pool(name="kp", bufs=NT) as kp,
            tc.tile_pool(name="vp", bufs=4 * NT) as vp,
            tc.tile_pool(name="vstage", bufs=3) as vstage,
            tc.tile_pool(name="pp", bufs=12) as pp,
            tc.tile_pool(name="rtmp", bufs=4) as rtmp,
            tc.tile_pool(name="ysb", bufs=6) as ysb,
            tc.tile_pool(name="rbp", bufs=2) as rbp,
            tc.tile_pool(name="osb", bufs=6) as osb,
            tc.tile_pool(name="mm_ps", bufs=3, space="PSUM") as mm_ps,
            tc.tile_pool(name="s_ps", bufs=2, space="PSUM") as s_ps,
            tc.tile_pool(name="y_ps", bufs=2, space="PSUM") as y_ps,
            tc.tile_pool(name="aux_ps", bufs=1, space="PSUM") as aux_ps,
        ):
            # ---- persistent tiles (DMA order matters: the very first qkv
            # matmuls need wa chunk 0 + x chunk 0; cos/sin follow for RoPE;
            # wp is not needed until the first out-projection) ----
            wa_sb = singles.tile([128, NCT, F], BF16)
            xt0 = xp.tile([128, NCT, TT], BF16, tag="xt")
            # first c-chunks as small separate transfers on two queues so the
            # first qkv matmul can start within a couple of microseconds
            nc.sync.dma_start(wa_sb[:, 0:2, :], waT_r[:, 0:2, :])
            nc.scalar.dma_start(xt0[:, 0:2, :], xT_r[:, 0:2, 0:TT])
            nc.sync.dma_start(wa_sb[:, 2:4, :], waT_r[:, 2:4, :])
            nc.scalar.dma_start(xt0[:, 2:4, :], xT_r[:, 2:4, 0:TT])
            for q in range(1, 4):
                nc.sync.dma_start(
                    wa_sb[:, 4 * q:4 * (q + 1), :], waT_r[:, 4 * q:4 * (q + 1), :]
                )
                nc.scalar.dma_start(
                    xt0[:, 4 * q:4 * (q + 1), :],
                    xT_r[:, 4 * q:4 * (q + 1), 0:TT],
                )
            cos_sb = singles.tile([128, T], FP32)
            nc.scalar.dma_start(cos_sb, cos2)
            sin_sb = singles.tile([128, T], FP32)
            nc.scalar.dma_start(sin_sb, sin2)
            wp_sb = singles.tile([128, 2, C], BF16)
            nc.gpsimd.dma_start(wp_sb, wpT_r)
            ident = singles.tile([128, 128], BF16)
            make_identity(nc, ident)
            ones_sb = singles.tile([128, 1], BF16)
            nc.vector.memset(ones_sb, 1.0)
            F32R = dt.float32r
            ones_colf = singles.tile([1, 128], FP32)
            nc.vector.memset(ones_colf, 1.0)
            ones_col = singles.tile([1, 128], F32R)
            nc.vector.tensor_copy(ones_col, ones_colf)

            q_tiles = [[None] * NT for _ in range(2)]
            k_tiles = [None] * NT
            v_tiles = [None] * (4 * NT)
            y_chunks = [[] for _ in range(NT)]

            def emit_proj(i):
                # out projection for t-chunk i; emitted one t-tile late so
                # attention matmuls are available to fill PE stalls while the
                # DVE/ACT drain copies pace the psum slot rotation
                for oi in range(NCT):
                    op = mm_ps.tile([128, TT], FP32, tag="mm")
                    for cj in range(2):
                        nc.tensor.matmul(
                            op,
                            wp_sb[:, cj, oi * 128:(oi + 1) * 128],
                            y_chunks[i][cj],
                            start=(cj == 0),
                            stop=(cj == 1),
                        )
                    ot = osb.tile([128, TT], FP32, tag="ot")
                    if oi % 2 == 0:
                        nc.vector.tensor_copy(ot, op)
                    else:
                        nc.scalar.copy(ot, op)
                    nc.sync.dma_start(
                        outT[oi * 128:(oi + 1) * 128, ts(i, TT)], ot
                    )

            for i in range(NT):
                # ---- QKV projection for t-chunk i ----
                if i == 0:
                    xt = xt0
                else:
                    xt = xp.tile([128, NCT, TT], BF16, tag="xt")
                    for q in range(4):
                        nc.sync.dma_start(
                            xt[:, 4 * q:4 * (q + 1), :],
                            xT_r[:, 4 * q:4 * (q + 1), ts(i, TT)],
                        )
                for f in range(4):
                    ps = mm_ps.tile([128, TT], FP32, tag="mm")
                    for ci in range(NCT):
                        nc.tensor.matmul(
                            ps,
                            wa_sb[:, ci, f * 128:(f + 1) * 128],
                            xt[:, ci, :],
                            start=(ci == 0),
                            stop=(ci == NCT - 1),
                        )
                    if taps:
                        dbg = osb.tile([128, TT], FP32, tag="dbg")
                        nc.vector.tensor_copy(dbg, ps)
                        nc.sync.dma_start(
                            d_qkvT[f * 128:(f + 1) * 128, ts(i, TT)], dbg
                        )
                    if f < 2:
                        dst = qp.tile([128, TT], BF16, tag="qt")
                        q_tiles[f][i] = dst
                        _rope(nc, rtmp, cos_sb[:, ts(i, TT)],
                              sin_sb[:, ts(i, TT)], ps, dst)
                        if taps:
                            nc.sync.dma_start(
                                d_QT[f * 128:(f + 1) * 128, ts(i, TT)], dst
                            )
                    elif f == 2:
                        dst = kp.tile([128, TT], BF16, tag="kt")
                        k_tiles[i] = dst
                        _rope(nc, rtmp, cos_sb[:, ts(i, TT)],
                              sin_sb[:, ts(i, TT)], ps, dst)
                        if taps:
                            nc.sync.dma_start(d_KT[:, ts(i, TT)], dst)
                    else:
                        vst = vstage.tile([128, TT], BF16, tag="vst")
                        nc.vector.tensor_copy(vst, ps)
                        for j4 in range(4):
                            tp = mm_ps.tile([128, 128], BF16, tag="mm")
                            nc.tensor.transpose(
                                tp, vst[:, j4 * 128:(j4 + 1) * 128], ident
                            )
                            vt = vp.tile([128, 128], BF16, tag="vt")
                            v_tiles[i * 4 + j4] = vt
                            nc.vector.tensor_copy(vt, tp)
                            if taps:
                                nc.sync.dma_start(
                                    d_V[(i * 4 + j4) * 128:(i * 4 + j4 + 1) * 128, :],
                                    vt,
                                )

                # ---- attention for t-chunk i, both heads ----
                yts = y_chunks[i]
                ns = 4 * (i + 1)
                for h in range(2):
                    yp = y_ps.tile([128, TT], FP32, tag="y")
                    dp = aux_ps.tile([1, TT], FP32, tag="den")
                    for j in range(ns):
                        sp = s_ps.tile([128, TT], FP32, tag="s")
                        nc.tensor.matmul(
                            sp,
                            k_tiles[j // 4][:, (j % 4) * 128:(j % 4 + 1) * 128],
                            q_tiles[h][i],
                            start=True,
                            stop=True,
                        )
                        p_sb = pp.tile([128, TT], BF16, tag="p")
                        nc.scalar.activation(p_sb, sp, AF.Exp, scale=SCALE)
                        if j >= 4 * i:
                            # diagonal block: zero entries with s > t, i.e.
                            # keep iff (t0 + y) - (s0 + p) >= 0; the iota is
                            # base + channel_multiplier*p + step*y, compared
                            # against 0 by compare_op.
                            nc.gpsimd.affine_select(
                                out=p_sb,
                                in_=p_sb,
                                compare_op=ALU.is_ge,
                                fill=0.0,
                                base=i * TT - j * 128,
                                pattern=[[1, TT]],
                                channel_multiplier=-1,
                            )
                        nc.tensor.matmul(
                            yp, v_tiles[j], p_sb,
                            start=(j == 0), stop=(j == ns - 1),
                        )
                        nc.tensor.matmul(
                            dp, ones_sb, p_sb,
                            start=(j == 0), stop=(j == ns - 1),
                        )
                        if taps and h == 0 and i == NT - 1:
                            nc.sync.dma_start(
                                d_P[j * 128:(j + 1) * 128, :], p_sb
                            )
                    # softmax denominator -> broadcast to 128 rows via a tiny
                    # K=1 fp32r matmul on PE (gpsimd partition_broadcast is
                    # ~1.7us + heavy sem cost on this path), then reciprocal
                    drow = rbp.tile([1, TT], F32R, tag="drow")
                    nc.vector.tensor_copy(drow, dp)
                    # the den bank is free once drow is drained, so the
                    # broadcast matmul reuses it instead of an mm slot
                    bc = aux_ps.tile([128, TT], FP32, tag="den")
                    nc.tensor.matmul(bc, ones_col, drow, start=True, stop=True)
                    rb = rbp.tile([128, TT], FP32, tag="rb")
                    nc.vector.reciprocal_approx_fast(out=rb, in_=bc)
                    yt = ysb.tile([128, TT], BF16, tag="yt")
                    nc.vector.tensor_mul(yt, yp, rb)
                    yts.append(yt)
                    if taps:
                        nc.sync.dma_start(d_den[h:h + 1, ts(i, TT)], drow)
                        nc.sync.dma_start(
                            d_y[h * 128:(h + 1) * 128, ts(i, TT)], yt
                        )

                if i > 0:
                    emit_proj(i - 1)
            emit_proj(NT - 1)

    nc.compile()
    return nc


def _prep_inputs(x, cos, sin, W_attn, W_proj):
    bf = ml_dtypes.bfloat16
    x = np.asarray(x, dtype=np.float32)
    cos = np.asarray(cos, dtype=np.float32)
    sin = np.asarray(sin, dtype=np.float32)
    W_attn = np.asarray(W_attn, dtype=np.float32)
    W_proj = np.asarray(W_proj, dtype=np.float32)

    xT = np.ascontiguousarray(x.reshape(T, C).T).astype(bf)
    cos2 = np.ascontiguousarray(np.concatenate([cos.T, cos.T], axis=0))
    sin2 = np.ascontiguousarray(np.concatenate([sin.T, sin.T], axis=0))

    in_maps = []
    for core in range(8):
        g = core // 2
        qoff = g * 768 + (core % 2) * 256
        rows = np.concatenate(
            [
                W_attn[qoff:qoff + 256],
                W_attn[g * 768 + 512:g * 768 + 640],
                W_attn[g * 768 + 640:g * 768 + 768],
            ],
            axis=0,
        )
        waT = np.ascontiguousarray(rows.T).astype(bf)
        h0 = g * 4 + (core % 2) * 2
        wpT = np.ascontiguousarray(W_proj[:, h0 * 128:h0 * 128 + 256].T).astype(bf)
        in_maps.append(
            {"xT": xT, "waT": waT, "wpT": wpT, "cos2": cos2, "sin2": sin2}
        )
    return in_maps


def kernel(x, cos, sin, W_attn, W_proj, _trace=False, _trace_cores=None):
    if "nc" not in _cache:
        _cache["nc"] = build()
    nc = _cache["nc"]
    in_maps = _prep_inputs(x, cos, sin, W_attn, W_proj)
    kwargs = {}
    if _trace:
        install_ntff_hook_shim()
        kwargs = dict(trace=True, trace_cores=_trace_cores or [0])
    res = run_bass_kernel_spmd(nc, in_maps, core_ids=list(range(8)), **kwargs)
    acc = np.zeros((C, T), dtype=np.float32)
    for r in res.results:
        acc += r["outT"]
    out = np.ascontiguousarray(acc.T).reshape(1, T, C)
    _cache["last_results"] = res
    return out


# revision 58
# speedup vs baseline: 1.3512x; 1.0314x over previous
"""Trainium2 Bass kernel for nn_Block_15066745274698 (GQA attention block).

Computation (B=1, T=4096, C=2048, 16 heads x 128, 4 KV groups):
  qkv = x @ W_attn.T ; split q/k/v ; RoPE(q, k) ; causal GQA attention ;
  out = y @ W_proj.T

Sharding: head-parallel over 8 cores, 2 query heads + their KV group per
core. No collectives: each core computes a partial out^T (its 2 heads
pushed through the matching W_proj columns); the host sums the 8 partials.

Device layout (per core) is transpose-oriented so every matmul contracts
over the partition dim with zero on-device transposes of activations:
  qkv^T (f x t) = W_attn_slice^T.T @ x^T      [via lhsT = W_attn^T tiles]
  S^T   (s x t) = K^T.T @ Q^T                 [scores transposed]
  y^T   (d x t) = V.T @ exp(S^T)              [V transposed once on PE]
  out^T (o x t) = W_proj_slice^T.T @ y^T
Softmax: no max-subtraction (scores bounded ~ +-5), exp on ACT with fused
1/sqrt(128) scale, causal handled by block skipping + gpsimd affine_select
on diagonal blocks, denominators via ones-vector matmul + DMA broadcast +
fast reciprocal.
"""
import sys

sys.path.insert(0, "/opt/trn_rl_repo")
import types

import numpy as np
import ml_dtypes

import os

import concourse.bass as bass
import concourse.mybir as mybir
import concourse.tile as tile
from concourse import bacc
from concourse.bass import ts
from concourse.bass_utils import run_bass_kernel_spmd
from concourse.masks import make_identity

if os.environ.get("K_LDWOPT", "0") == "1":
    import concourse.bass_utils as _bu

    _orig_run_command = _bu.run_command

    def _patched_run_command(argv, **kwargs):
        argv = [
            a.replace("--enable-ldw-opt=false", "--enable-ldw-opt=true")
            if isinstance(a, str)
            else a
            for a in argv
        ]
        return _orig_run_command(argv, **kwargs)

    _bu.run_command = _patched_run_command

T, C = 4096, 2048
HS = 128
TT = 512                 # t-tile (matmul moving free dim)
NT = T // TT             # 8
NCT = C // 128           # 16 c-tiles
F = 512                  # per-core W_attn rows: 2 q heads + k + v
SCALE = 1.0 / float(np.sqrt(np.float32(HS)))

dt = mybir.dt
FP32 = dt.float32
BF16 = dt.bfloat16
AF = mybir.ActivationFunctionType
ALU = mybir.AluOpType

_cache = {}


def install_ntff_hook_shim():
    """antenv.axon_hooks is missing from this image; register the
    ctypes-based NTFF hook ourselves so trace=True works under axon."""
    if "antenv.axon_hooks" in sys.modules:
        return
    import antenv

    mod = types.ModuleType("antenv.axon_hooks")
    mod._hook = None
    mod.set_axon_ntff_profile_hook = lambda h: setattr(mod, "_hook", h)
    mod.get_axon_ntff_profile_hook = lambda: mod._hook
    sys.modules["antenv.axon_hooks"] = mod
    antenv.axon_hooks = mod
    try:
        from trn_agent_boot.trn_boot import _ntff_profile_via_ctypes

        mod.set_axon_ntff_profile_hook(
            _ntff_profile_via_ctypes("/opt/axon/libaxon_pjrt.so")
        )
    except Exception:
        pass


def _rope(nc, rtmp, cos_sl, sin_sl, src_ps, dst):
    """Rotate-half RoPE: src_ps (128d x TT) psum fp32 -> dst (128 x TT) bf16.
    cos_sl/sin_sl are (128 x TT) fp32 with the 64 rotary rows duplicated.
    Two-input DVE ops need equal base partitions only when BOTH inputs are
    SBUF; src_ps is PSUM, so the rotate-half partition shift is applied on
    the PSUM operand and all SBUF+SBUF pairs stay base-aligned."""
    tcos = rtmp.tile([128, TT], FP32, tag="tcos")
    tsin = rtmp.tile([128, TT], FP32, tag="tsin")
    nc.vector.tensor_mul(tcos, src_ps, cos_sl)
    nc.vector.tensor_mul(tsin[0:64, :], src_ps[64:128, :], sin_sl[0:64, :])
    nc.vector.tensor_mul(tsin[64:128, :], src_ps[0:64, :], sin_sl[64:128, :])
    nc.vector.tensor_sub(dst[0:64, :], tcos[0:64, :], tsin[0:64, :])
    nc.vector.tensor_add(dst[64:128, :], tcos[64:128, :], tsin[64:128, :])


def build(taps=False):
    nc = bacc.Bacc(
        "TRN2", target_bir_lowering=False, debug=False, enable_asserts=False
    )
    xT = nc.dram_tensor("xT", [C, T], BF16, kind="ExternalInput").ap()
    waT = nc.dram_tensor("waT", [C, F], BF16, kind="ExternalInput").ap()
    wpT = nc.dram_tensor("wpT", [2 * HS, C], BF16, kind="ExternalInput").ap()
    cos2 = nc.dram_tensor("cos2", [128, T], FP32, kind="ExternalInput").ap()
    sin2 = nc.dram_tensor("sin2", [128, T], FP32, kind="ExternalInput").ap()
    outT = nc.dram_tensor("outT", [C, T], FP32, kind="ExternalOutput").ap()
    if taps:
        d_qkvT = nc.dram_tensor("d_qkvT", [F, T], FP32, kind="ExternalOutput").ap()
        d_QT = nc.dram_tensor("d_QT", [256, T], BF16, kind="ExternalOutput").ap()
        d_KT = nc.dram_tensor("d_KT", [128, T], BF16, kind="ExternalOutput").ap()
        d_V = nc.dram_tensor("d_V", [T, 128], BF16, kind="ExternalOutput").ap()
        d_den = nc.dram_tensor("d_den", [2, T], FP32, kind="ExternalOutput").ap()
        d_y = nc.dram_tensor("d_y", [256, T], BF16, kind="ExternalOutput").ap()
        d_P = nc.dram_tensor("d_P", [T, TT], BF16, kind="ExternalOutput").ap()

    xT_r = xT.rearrange("(a p) t -> p a t", p=128)     # [128, 16, 4096]
    waT_r = waT.rearrange("(a p) f -> p a f", p=128)   # [128, 16, 512]
    wpT_r = wpT.rearrange("(a p) o -> p a o", p=128)   # [128, 2, 2048]

    with tile.TileContext(nc) as tc:
        with (
            tc.tile_pool(name="singles", bufs=1) as singles,
            tc.tile_pool(name="xp", bufs=3) as xp,
            tc.tile_pool(name="qp", bufs=2 * NT) as qp,
            tc.tile_pool(name="kp", bufs=NT) as kp,
            tc.tile_pool(name="vp", bufs=4 * NT) as vp,
            tc.tile_pool(name="vstage", bufs=3) as vstage,
            tc.tile_pool(name="pp", bufs=12) as pp,
            tc.tile_pool(name="rtmp", bufs=4) as rtmp,
            tc.tile_pool(name="ysb", bufs=6) as ysb,
            tc.tile_pool(name="rbp", bufs=2) as rbp,
            tc.tile_pool(name="osb", bufs=6) as osb,
            tc.tile_pool(name="mm_ps", bufs=3, space="PSUM") as mm_ps,
            tc.tile_pool(name="s_ps", bufs=2, space="PSUM") as s_ps,
            tc.tile_pool(name="y_ps", bufs=2, space="PSUM") as y_ps,
            tc.tile_pool(name="aux_ps", bufs=1, space="PSUM") as aux_ps,
        ):
            # ---- persistent tiles (DMA order matters: the very first qkv
            # matmuls need wa chunk 0 + x chunk 0; cos/sin follow for RoPE;
            # wp is not needed until the first out-projection) ----
            wa_sb = singles.tile([128, NCT, F], BF16)
            xt0 = xp.tile([128, NCT, TT], BF16, tag="xt")
            # first c-chunks as small separate transfers on two queues so the
            # first qkv matmul can start within a couple of microseconds
            nc.sync.dma_start(wa_sb[:, 0:2, :], waT_r[:, 0:2, :])
            nc.scalar.dma_start(xt0[:, 0:2, :], xT_r[:, 0:2, 0:TT])
            nc.sync.dma_start(wa_sb[:, 2:4, :], waT_r[:, 2:4, :])
            nc.scalar.dma_start(xt0[:, 2:4, :], xT_r[:, 2:4, 0:TT])
            for q in range(1, 4):
                nc.sync.dma_start(
                    wa_sb[:, 4 * q:4 * (q + 1), :], waT_r[:, 4 * q:4 * (q + 1), :]
                )
                nc.scalar.dma_start(
                    xt0[:, 4 * q:4 * (q + 1), :],
                    xT_r[:, 4 * q:4 * (q + 1), 0:TT],
                )
            cos_sb = singles.tile([128, T], FP32)
            nc.scalar.dma_start(cos_sb, cos2)
            sin_sb = singles.tile([128, T], FP32)
            nc.scalar.dma_start(sin_sb, sin2)
            wp_sb = singles.tile([128, 2, C], BF16)
            nc.gpsimd.dma_start(wp_sb, wpT_r)
            ident = singles.tile([128, 128], BF16)
            make_identity(nc, ident)
            ones_sb = singles.tile([128, 1], BF16)
            nc.vector.memset(ones_sb, 1.0)
            F32R = dt.float32r
            ones_colf = singles.tile([1, 128], FP32)
            nc.vector.memset(ones_colf, 1.0)
            ones_col = singles.tile([1, 128], F32R)
            nc.vector.tensor_copy(ones_col, ones_colf)

            q_tiles = [[None] * NT for _ in range(2)]
            k_tiles = [None] * NT
            v_tiles = [None] * (4 * NT)
            y_chunks = [[] for _ in range(NT)]

            def emit_proj(i):
                # out projection for t-chunk i; emitted one t-tile late so
                # attention matmuls are available to fill PE stalls while the
                # DVE/ACT drain copies pace the psum slot rotation
                for oi in range(NCT):
                    op = mm_ps.tile([128, TT], FP32, tag="mm")
                    for cj in range(2):
                        nc.tensor.matmul(
                            op,
                            wp_sb[:, cj, oi * 128:(oi + 1) * 128],
                            y_chunks[i][cj],
                            start=(cj == 0),
                            stop=(cj == 1),
                        )
                    ot = osb.tile([128, TT], FP32, tag="ot")
                    if oi % 2 == 0:
                        nc.vector.tensor_copy(ot, op)
                    else:
                        nc.scalar.copy(ot, op)
                    nc.sync.dma_start(
                        outT[oi * 128:(oi + 1) * 128, ts(i, TT)], ot
                    )

            for i in range(NT):
                # ---- QKV projection for t-chunk i ----
                if i == 0:
                    xt = xt0
                else:
                    xt = xp.tile([128, NCT, TT], BF16, tag="xt")
                    for q in range(4):
                        nc.sync.dma_start(
                            xt[:, 4 * q:4 * (q + 1), :],
                            xT_r[:, 4 * q:4 * (q + 1), ts(i, TT)],
                        )
                for f in range(4):
                    ps = mm_ps.tile([128, TT], FP32, tag="mm")
                    for ci in range(NCT):
                        nc.tensor.matmul(
                            ps,
                            wa_sb[:, ci, f * 128:(f + 1) * 128],
                            xt[:, ci, :],
                            start=(ci == 0),
                            stop=(ci == NCT - 1),
                        )
                    if taps:
                        dbg = osb.tile([128, TT], FP32, tag="dbg")
                        nc.vector.tensor_copy(dbg, ps)
                        nc.sync.dma_start(
                            d_qkvT[f * 128:(f + 1) * 128, ts(i, TT)], dbg
                        )
                    if f < 2:
                        dst = qp.tile([128, TT], BF16, tag="qt")
                        q_tiles[f][i] = dst
                        _rope(nc, rtmp, cos_sb[:, ts(i, TT)],
                              sin_sb[:, ts(i, TT)], ps, dst)
                        if taps:
                            nc.sync.dma_start(
                                d_QT[f * 128:(f + 1) * 128, ts(i, TT)], dst
                            )
                    elif f == 2:
                        dst = kp.tile([128, TT], BF16, tag="kt")
                        k_tiles[i] = dst
                        _rope(nc, rtmp, cos_sb[:, ts(i, TT)],
                              sin_sb[:, ts(i, TT)], ps, dst)
                        if taps:
                            nc.sync.dma_start(d_KT[:, ts(i, TT)], dst)
                    else:
                        vst = vstage.tile([128, TT], BF16, tag="vst")
                        nc.vector.tensor_copy(vst, ps)
                        for j4 in range(4):
                            tp = mm_ps.tile([128, 128], BF16, tag="mm")
                            nc.tensor.transpose(
                                tp, vst[:, j4 * 128:(j4 + 1) * 128], ident
                            )
                            vt = vp.tile([128, 128], BF16, tag="vt")
                            v_tiles[i * 4 + j4] = vt
                            nc.vector.tensor_copy(vt, tp)
                            if taps:
                                nc.sync.dma_start(
                                    d_V[(i * 4 + j4) * 128:(i * 4 + j4 + 1) * 128, :],
                                    vt,
                                )

                # ---- attention for t-chunk i, both heads ----
                yts = y_chunks[i]
                ns = 4 * (i + 1)
                for h in range(2):
                    yp = y_ps.tile([128, TT], FP32, tag="y")
                    dp = aux_ps.tile([1, TT], FP32, tag="den")
                    for j in range(ns):
                        # diagonal s-tiles: only the causally-valid column
                        # suffix [off:TT) is computed (off = s0 - t0); the
                        # j == 0 matmul always has off == 0, so every psum
                        # column is initialized by the start=True group head
                        off = (j % 4) * 128 if j >= 4 * i else 0
                        nv = TT - off
                        sp = s_ps.tile([128, TT], FP32, tag="s")
                        nc.tensor.matmul(
                            sp[:, off:],
                            k_tiles[j // 4][:, (j % 4) * 128:(j % 4 + 1) * 128],
                            q_tiles[h][i][:, off:],
                            start=True,
                            stop=True,
                        )
                        p_sb = pp.tile([128, TT], BF16, tag="p")
                        nc.scalar.activation(
                            p_sb[:, off:], sp[:, off:], AF.Exp, scale=SCALE
                        )
                        if j >= 4 * i:
                            # zero entries with s > t inside the aligned
                            # 128-wide triangle at the start of the slice:
                            # keep iff y - p >= 0 (base 0 after slicing)
                            nc.gpsimd.affine_select(
                                out=p_sb[:, off:],
                                in_=p_sb[:, off:],
                                compare_op=ALU.is_ge,
                                fill=0.0,
                                base=0,
                                pattern=[[1, nv]],
                                channel_multiplier=-1,
                            )
                        nc.tensor.matmul(
                            yp[:, off:], v_tiles[j], p_sb[:, off:],
                            start=(j == 0), stop=(j == ns - 1),
                            skip_group_check=True,
                        )
                        nc.tensor.matmul(
                            dp[:, off:], ones_sb, p_sb[:, off:],
                            start=(j == 0), stop=(j == ns - 1),
                            skip_group_check=True,
                        )
                        if taps and h == 0 and i == NT - 1:
                            nc.sync.dma_start(
                                d_P[j * 128:(j + 1) * 128, :], p_sb
                            )
                    # softmax denominator -> broadcast to 128 rows via a tiny
                    # K=1 fp32r matmul on PE (gpsimd partition_broadcast is
                    # ~1.7us + heavy sem cost on this path), then reciprocal
                    drow = rbp.tile([1, TT], F32R, tag="drow")
                    nc.vector.tensor_copy(drow, dp)
                    # the den bank is free once drow is drained, so the
                    # broadcast matmul reuses it instead of an mm slot
                    bc = aux_ps.tile([128, TT], FP32, tag="den")
                    nc.tensor.matmul(bc, ones_col, drow, start=True, stop=True)
                    rb = rbp.tile([128, TT], FP32, tag="rb")
                    nc.vector.reciprocal_approx_fast(out=rb, in_=bc)
                    yt = ysb.tile([128, TT], BF16, tag="yt")
                    nc.vector.tensor_mul(yt, yp, rb)
                    yts.append(yt)
                    if taps:
                        nc.sync.dma_start(d_den[h:h + 1, ts(i, TT)], drow)
                        nc.sync.dma_start(
                            d_y[h * 128:(h + 1) * 128, ts(i, TT)], yt
                        )

                if i > 0:
                    emit_proj(i - 1)
            emit_proj(NT - 1)

    nc.compile()
    return nc


def _prep_inputs(x, cos, sin, W_attn, W_proj):
    bf = ml_dtypes.bfloat16
    x = np.asarray(x, dtype=np.float32)
    cos = np.asarray(cos, dtype=np.float32)
    sin = np.asarray(sin, dtype=np.float32)
    W_attn = np.asarray(W_attn, dtype=np.float32)
    W_proj = np.asarray(W_proj, dtype=np.float32)

    xT = np.ascontiguousarray(x.reshape(T, C).T).astype(bf)
    cos2 = np.ascontiguousarray(np.concatenate([cos.T, cos.T], axis=0))
    sin2 = np.ascontiguousarray(np.concatenate([sin.T, sin.T], axis=0))

    in_maps = []
    for core in range(8):
        g = core // 2
        qoff = g * 768 + (core % 2) * 256
        rows = np.concatenate(
            [
                W_attn[qoff:qoff + 256],
                W_attn[g * 768 + 512:g * 768 + 640],
                W_attn[g * 768 + 640:g * 768 + 768],
            ],
            axis=0,
        )
        waT = np.ascontiguousarray(rows.T).astype(bf)
        h0 = g * 4 + (core % 2) * 2
        wpT = np.ascontiguousarray(W_proj[:, h0 * 128:h0 * 128 + 256].T).astype(bf)
        in_maps.append(
            {"xT": xT, "waT": waT, "wpT": wpT, "cos2": cos2, "sin2": sin2}
        )
    return in_maps


def kernel(x, cos, sin, W_attn, W_proj, _trace=False, _trace_cores=None):
    if "nc" not in _cache:
        _cache["nc"] = build()
    nc = _cache["nc"]
    in_maps = _prep_inputs(x, cos, sin, W_attn, W_proj)
    kwargs = {}
    if _trace:
        install_ntff_hook_shim()
        kwargs = dict(trace=True, trace_cores=_trace_cores or [0])
    res = run_bass_kernel_spmd(nc, in_maps, core_ids=list(range(8)), **kwargs)
    acc = np.zeros((C, T), dtype=np.float32)
    for r in res.results:
        acc += r["outT"]
    out = np.ascontiguousarray(acc.T).reshape(1, T, C)
    _cache["last_results"] = res
    return out
